# revision 1
# baseline (speedup 1.0000x reference)
"""Trainium2 Bass kernel for nn_ConvGraphQNN (gnn_message_passing).

Reference semantics:
    f = sigmoid(unfold(x, k=2) @ W.T + b)            # [B, L] node feats, dim 1
    nf = f / (|f| + 1e-12)  (f > 0, so nf = f/(f+1e-12))
    sim = nf nf^T ; w = (sim >= 0.9) minus diagonal
    out_b = mean_i [ f_i + (w @ f)_i / row_sum(w)_i ]

Because the node feature dim is 1, whenever min(f) >= 1e-9 every nf >= 0.999
so every off-diagonal sim >= 0.998 > 0.9: the adjacency is exactly the
complete graph, row sums are L-1, and

    out_b = mean_i [ f_i + (S - f_i)/(L-1) ] = 2 * S / L,   S = sum(f).

(The threshold could only fail if some sigmoid output were < ~2e-11, i.e. a
logit below -24.6; min(f) is checked via an on-device row-min of the conv
logits and a full host fallback is used if it ever fails.)

Device work per core (8 cores, SPMD): a 24-row slice of the 95x95 conv
output grid for one batch image. Raw Bass (no TileContext): one packed
input DMA, the 4-tap conv as fused vector multiply-adds, a row-min of the
logits (guard), sigmoid with fused row-sum on the scalar engine (bias
rides the activation), and the scalar engine itself issues the [24, 2]
output DMA. Cross-engine sync is explicit standalone semaphore waits
(this toolchain's walrus codegen only encodes one embedded wait per
instruction). The Bass-init all-engine barrier only guards unused
const-AP memsets, so it is stripped; all ordering here is via explicit
semaphores. The host combines 190 row sums per batch into [B, 1].
"""

import sys

for _p in ("/opt/trn_rl_repo", "/opt/pypackages"):
    if _p not in sys.path:
        sys.path.append(_p)

import numpy as np

import concourse.bass as bass
import concourse.mybir as mybir
from concourse.bass_utils import run_bass_kernel_spmd

KS = 2
HI = 96          # input H = W
HO = 95          # conv output H = W (stride 1, k 2)
L = HO * HO      # 9025 nodes per graph
B = 2
N_CORES = 8
R = 24           # output rows per core (uniform SPMD program)
STARTS = [0, 24, 48, 71]   # row starts per quarter; q=3 overlaps row 71,
                           # host drops its first row
PKW = 2 * HI + 5           # packed input: x0 | x1 | [w00 w01 w10 w11 b]
GRAPH_T = 0.9
GUARD_MIN_F = 1e-9

_CACHE = {}


def _build_bass():
    if "nc" in _CACHE:
        return _CACHE["nc"]
    nc = _trace_bass()
    try:
        _strip_init_barrier(nc)
    except AssertionError:
        # Structure drifted from what the surgery expects — fall back to
        # the untouched (slower but correct) program.
        nc = _trace_bass()
    _CACHE["nc"] = nc
    return nc


def _trace_bass():
    fp32 = mybir.dt.float32
    mult = mybir.AluOpType.mult
    add = mybir.AluOpType.add

    nc = bass.Bass("TRN2")
    pk = nc.dram_tensor("pk", [R, PKW], fp32, kind="ExternalInput")
    o = nc.dram_tensor("o", [R, 2], fp32, kind="ExternalOutput")
    with (
        nc.sbuf_tensor([R, PKW], fp32) as PK,
        nc.sbuf_tensor([R, HO], fp32) as ACC,
        nc.sbuf_tensor([R, HO], fp32) as F,
        nc.sbuf_tensor([R, 2], fp32) as OUT,
        nc.semaphore() as dsem,
        nc.semaphore() as vsem,
        nc.semaphore() as asem,
        nc.Block() as block,
    ):
        X0 = PK[:, 0:HI]
        X1 = PK[:, HI:2 * HI]
        WB = PK[:, 2 * HI:2 * HI + 5]

        @block.sync
        def _(sync):
            sync.dma_start(out=PK[:, :], in_=pk[:, :]).then_inc(dsem, 16)
            # SP also issues the output DMA: its DGE path is ~230ns cheaper
            # than ScalarE's (dge delay 650 vs 784, seq 565 vs 667), more
            # than the one extra cross-engine sem handoff costs.
            sync.wait_ge(vsem, 2)    # DVE min landed in OUT[:, 1]
            sync.wait_ge(asem, 1)    # ScalarE row-sums landed in OUT[:, 0]
            sync.dma_start(out=o[:, :], in_=OUT[:, :]).then_inc(dsem, 16)
            sync.wait_ge(dsem, 32)   # out-DMA landed before NEFF end

        @block.vector
        def _(vector):
            vector.wait_ge(dsem, 16)
            # acc = w00*x[r,c] + w01*x[r,c+1] + w10*x[r+1,c] + w11*x[r+1,c+1]
            nc.vector.tensor_scalar(
                out=ACC[:, :], in0=X0[:, 0:HO],
                scalar1=WB[:, 0:1], scalar2=None, op0=mult)
            nc.vector.scalar_tensor_tensor(
                out=ACC[:, :], in0=X0[:, 1:HI], scalar=WB[:, 1:2],
                in1=ACC[:, :], op0=mult, op1=add)
            nc.vector.scalar_tensor_tensor(
                out=ACC[:, :], in0=X1[:, 0:HO], scalar=WB[:, 2:3],
                in1=ACC[:, :], op0=mult, op1=add)
            nc.vector.scalar_tensor_tensor(
                out=ACC[:, :], in0=X1[:, 1:HI], scalar=WB[:, 3:4],
                in1=ACC[:, :], op0=mult, op1=add).then_inc(vsem, 1)
            # per-row min of the pre-bias conv; sigmoid is monotonic so the
            # host recovers min(f) = sigmoid(min + b) for the guard.
            nc.vector.tensor_reduce(
                out=OUT[:, 1:2], in_=ACC[:, :],
                axis=mybir.AxisListType.X,
                op=mybir.AluOpType.min).then_inc(vsem, 1)

        @block.scalar
        def _(scalar):
            # vsem>=1 transitively implies dsem>=16 (DVE waited on the DMA
            # before producing ACC), so no separate dsem wait for WB here.
            scalar.wait_ge(vsem, 1)    # ACC ready
            # f = sigmoid(acc + b) with fused per-row sum into OUT[:, 0]
            nc.scalar.activation(
                out=F[:, :], in_=ACC[:, :],
                func=mybir.ActivationFunctionType.Sigmoid,
                bias=WB[:, 4:5], scale=1.0,
                accum_out=OUT[:, 0:1]).then_inc(asem, 1)

    return nc


def _strip_init_barrier(nc):
    """Post-trace edits.

    1. Bass.__init__ emits const-AP memsets plus an all-engine barrier
       before the kernel body. Nothing here reads the const APs and all
       cross-engine ordering is explicit semaphores, so drop the barrier
       (Drain + EventSemaphore per engine).
    2. Hoist the input DMACopy ahead of SP's five prelude RegisterMoves
       (zero/bounds-reg init). The DMA references no registers, so the
       moves can run during the transfer instead of serializing ~250ns
       before it on the critical path.
    3. Drop the Block-exit all-engine barrier (the *_end block). SP's
       final dsem>=32 wait already guarantees every DMA landed and every
       engine finished before the NEFF completes. Semaphore state was
       probed to reset between executions on this runtime (4 consecutive
       device runs with different inputs all correct), so no tail
       clears/barrier are needed for re-execution."""
    blocks = nc.m.functions[0].blocks
    bb0 = blocks[0]
    keep, removed = [], []
    for ins in bb0.instructions:
        tn = type(ins).__name__
        if "Drain" in tn or "EventSemaphore" in tn or \
                ins.name.startswith("barrier_"):
            removed.append(ins.name)
            continue
        keep.append(ins)
    assert len(removed) >= 10, removed   # 5 engines x (drain + evsem)

    in_dma = None
    for blk in blocks[1:]:
        for ins in blk.instructions:
            if type(ins).__name__.endswith("InstDMACopy") or \
                    "DMACopy" in type(ins).__name__:
                src = ins.ins[0]
                if getattr(src, "memref", "") == "pk":
                    in_dma = ins
                    blk.instructions = [
                        i for i in blk.instructions if i.name != ins.name]
                    break
        if in_dma is not None:
            break
    assert in_dma is not None, "input DMACopy not found"
    # index 0 is the pseudo Call; engines only order among their own stream
    bb0.instructions = keep[:1] + [in_dma] + keep[1:]

    end_blk = None
    for blk in blocks:
        if blk.name.endswith("_end"):
            assert all(
                "Drain" in type(i).__name__ or
                "EventSemaphore" in type(i).__name__
                for i in blk.instructions), [
                    type(i).__name__ for i in blk.instructions]
            blk.instructions = []
            end_blk = blk
    assert end_blk is not None, "Block end bb not found"

    # 4. Move SP's final dsem wait past its branch, into the end block —
    #    otherwise the 50ns branch retires after the wait and tail-pads
    #    the kernel. Per-engine order is preserved: SP runs the body,
    #    branches to the end block, and waits there.
    for blk in blocks:
        insts = blk.instructions
        if any("DMACopy" in type(i).__name__ and
               getattr(i.outs[0], "memref", "") == "o" for i in insts):
            assert "EventSemaphore" in type(insts[-2]).__name__ and \
                "UnconditionalBranch" in type(insts[-1]).__name__, [
                    type(i).__name__ for i in insts[-2:]]
            w = insts[-2].sync_info.on_wait
            assert len(w) == 1 and w[0].wait_value == 32, w
            final_wait = insts[-2]
            blk.instructions = insts[:-2] + insts[-1:]
            end_blk.instructions = [final_wait]
            break
    else:
        raise AssertionError("SP body block with output DMA not found")


def _in_maps(x, W, b):
    wb_row = np.concatenate([W.reshape(-1), b.reshape(-1)]).astype(np.float32)
    maps = []
    for c in range(N_CORES):
        bi, s = c // 4, STARTS[c % 4]
        pk = np.empty((R, PKW), dtype=np.float32)
        pk[:, 0:HI] = x[bi, 0, s:s + R, :]
        pk[:, HI:2 * HI] = x[bi, 0, s + 1:s + R + 1, :]
        pk[:, 2 * HI:] = wb_row[None, :]
        maps.append({"pk": pk})
    return maps


def _run_device(x, W, b, trace=False, **kw):
    nc = _build_bass()
    res = run_bass_kernel_spmd(
        nc, _in_maps(x, W, b), core_ids=list(range(N_CORES)), trace=trace, **kw
    )
    return res


def _combine(results, b):
    """results: 8 dicts of o [R,2] -> ([B,1] out, global min f)."""
    out = np.zeros((B, 1), dtype=np.float32)
    gmin_acc = np.inf
    for bi in range(B):
        sums = []
        for q in range(4):
            r = results[bi * 4 + q]
            s = r["o"][:, 0]
            m = r["o"][:, 1]
            if q == 3:          # row 71 is also computed by q=2; drop dup
                s = s[1:]
                m = m[1:]
            sums.append(s)
            gmin_acc = min(gmin_acc, float(m.min()))
        S = float(np.concatenate(sums).astype(np.float64).sum())
        out[bi, 0] = np.float32(2.0 * S / L)
    # sigmoid is monotonic: min f = sigmoid(min conv + b)
    ga = gmin_acc + float(np.asarray(b).reshape(-1)[0])
    gmin_f = 1.0 / (1.0 + np.exp(-ga))
    return out, gmin_f


def _fallback(x, W, b):
    # Exact O(L log L) host evaluation of the reference semantics; only
    # reached if some sigmoid output underflows below GUARD_MIN_F.
    out = np.zeros((B, 1), dtype=np.float32)
    W4 = W.reshape(-1).astype(np.float64)
    for bi in range(B):
        img = x[bi, 0].astype(np.float64)
        acc = (W4[0] * img[:-1, :-1] + W4[1] * img[:-1, 1:]
               + W4[2] * img[1:, :-1] + W4[3] * img[1:, 1:]) + float(b[0])
        f = (1.0 / (1.0 + np.exp(-acc))).reshape(-1)
        nf = f / (f + 1e-12)
        order = np.argsort(nf)
        nf_s, f_s = nf[order], f[order]
        suff_f = np.cumsum(f_s[::-1])[::-1]
        thr = GRAPH_T / nf
        idx = np.searchsorted(nf_s, thr, side="left")
        cnt = (len(f) - idx).astype(np.float64)
        aggs = np.where(idx < len(f), suff_f[np.minimum(idx, len(f) - 1)], 0.0)
        self_in = nf * nf >= GRAPH_T
        cnt = cnt - self_in
        aggs = aggs - np.where(self_in, f, 0.0)
        node = f + np.where(cnt > 0, aggs / np.maximum(cnt, 1), 0.0)
        out[bi, 0] = np.float32(node.mean())
    return out


def kernel(x, W, b):
    x = np.ascontiguousarray(np.asarray(x, dtype=np.float32))
    W = np.asarray(W, dtype=np.float32)
    b = np.asarray(b, dtype=np.float32)
    res = _run_device(x, W, b, trace=False)
    out, gmin = _combine(res.results, b)
    if not (gmin >= GUARD_MIN_F):
        return _fallback(x, W, b)
    return out



# revision 7
# speedup vs baseline: 1.3941x; 1.3941x over previous
"""Trainium2 Bass kernel for nn_ConvGraphQNN (gnn_message_passing).

Reference semantics:
    f = sigmoid(unfold(x, k=2) @ W.T + b)            # [B, L] node feats, dim 1
    nf = f / (|f| + 1e-12)  (f > 0, so nf = f/(f+1e-12))
    sim = nf nf^T ; w = (sim >= 0.9) minus diagonal
    out_b = mean_i [ f_i + (w @ f)_i / row_sum(w)_i ]

Because the node feature dim is 1, whenever min(f) >= 1e-9 every nf >= 0.999
so every off-diagonal sim >= 0.998 > 0.9: the adjacency is exactly the
complete graph, row sums are L-1, and

    out_b = mean_i [ f_i + (S - f_i)/(L-1) ] = 2 * S / L,   S = sum(f).

(The threshold could only fail if some sigmoid output were < ~2e-11, i.e. a
logit below -24.6; min(logit+b) is tracked via an on-device reduce and a full
host fallback is used if it ever fails.)

Device work per core (8 cores, SPMD): a 24-column slice of the 95x95 conv
output grid for one batch image, laid out TRANSPOSED: partition = output row
(95 partitions), free dim = the core's 24 columns. The free dim is 4x smaller
than a row-sliced layout, which cuts every DVE/ACT instruction's per-element
time; the row pair x[r], x[r+1] is duplicated per partition because engine
access patterns only allow partition starts of 0/32/64/96 (no +1 partition
shifts). The 4-tap conv is tensor_scalar (2x_2p fast path, bias folded via
scalar2) + 3 scalar_tensor_tensor; sigmoid+row-sum fused on ACT (accum_out);
per-partition min of the logits rides DVE for the guard.

The [128,2] result is stored via a PREPARED SWDGE kv_writeback: Pool
generates the descriptors at t=0 (994ns, fully hidden behind the input DMA)
and trigger_dma fires them when ACT's semaphore lands — skipping the
625ns HWDGE + 650ns DGE delay a fresh DMACopy would pay at that point.
All cross-engine waits are embedded (one per instruction, the walrus limit)
so they park at ENGINE level and never stall a sequencer.

Because 95 = 4*24 - 1, column 71 is computed by two cores; the host
subtracts its sigmoid-sum once (95 sigmoids per image in numpy) during the
combine. The host combines 8 cores' column sums into [B, 1].
"""

import sys

for _p in ("/opt/trn_rl_repo", "/opt/pypackages"):
    if _p not in sys.path:
        sys.path.append(_p)

import numpy as np

import concourse.bass as bass
import concourse.mybir as mybir
from concourse.bass_utils import run_bass_kernel_spmd

HI = 96          # input H = W
HO = 95          # conv output H = W (stride 1, k 2)
L = HO * HO      # 9025 nodes per graph
B = 2
N_CORES = 8
CW = 24          # output columns per core (free dim; uniform SPMD program)
STARTS = [0, 24, 48, 71]   # column starts per quarter; q=3 overlaps col 71,
                           # host subtracts its sigmoid-sum once
PKW = 55         # packed per-partition: xA(25) | xB(25) | [w00 w01 w10 w11 b]
GRAPH_T = 0.9
GUARD_MIN_F = 1e-9

_CACHE = {}


def _build_bass():
    if "nc" in _CACHE:
        return _CACHE["nc"]
    nc = _trace_bass()
    # Raw Bass skips the Bacc pass that packs .instr bytes for InstISA
    # subclasses (here: InstTriggerDma); without it walrus codegen fails
    # with "ISA wrong length".
    mybir.codegen_inst_isa_subclasses(nc)
    try:
        _strip_init_barrier(nc)
    except AssertionError:
        # Structure drifted from what the surgery expects — fall back to
        # the untouched (slower but correct) program.
        nc = _trace_bass()
        mybir.codegen_inst_isa_subclasses(nc)
    _CACHE["nc"] = nc
    return nc


def _trace_bass():
    fp32 = mybir.dt.float32
    i32 = mybir.dt.int32
    mult = mybir.AluOpType.mult
    add = mybir.AluOpType.add

    nc = bass.Bass("TRN2")
    pk = nc.dram_tensor("pk", [HO, PKW], fp32, kind="ExternalInput")
    o = nc.dram_tensor("o", [1, 128, 1, 2], fp32, kind="ExternalOutput")
    with (
        nc.sbuf_tensor([HO, PKW], fp32) as PK,
        nc.sbuf_tensor([HO, CW], fp32) as ACC,
        nc.sbuf_tensor([HO, CW], fp32) as F,
        nc.sbuf_tensor([128, 1, 1, 2], fp32) as OUT,
        nc.sbuf_tensor([128, 1], i32) as IDX,
        nc.semaphore() as dsem,
        nc.semaphore() as vsem,
        nc.semaphore() as asem,
        nc.semaphore() as ksem,
        nc.semaphore() as psem,
        nc.Block() as block,
    ):
        XA = PK[:, 0:25]          # x[r,   C..C+24]
        XB = PK[:, 25:50]         # x[r+1, C..C+24]
        WB = PK[:, 50:55]         # w00 w01 w10 w11 b (replicated per row)

        @block.sync
        def _(sync):
            sync.dma_start(out=PK[:, :], in_=pk[:, :]).then_inc(dsem, 16)
            # Execution-safety tail: ensure the triggered output DMA landed
            # before any engine stream can retire the NEFF.
            sync.wait_ge(ksem, 16)

        @block.vector
        def _(vector):
            # OUT rows 95..127 are transferred but unused; zero them so the
            # DMA never reads uninitialized SBUF. Must live on DVE (not
            # Pool): Pool's slow load_library would otherwise delay this
            # memset past the ACT/DVE result writes and clobber them.
            nc.vector.memset(OUT[:, :, :, :], 0)
            # Standalone wait: an embedded wait would park only this
            # instruction while the dependent stt ops behind it BYPASS the
            # wait queue on hardware and read garbage (verified on this
            # runtime); a sequencer-level wait orders the whole stream.
            vector.wait_ge(dsem, 16)
            # acc = w00*x[r,c+j] + b   (tensor_scalar 2x_2p fast path; the
            # bias rides scalar2 so ACT's bias stays 0)
            nc.vector.tensor_scalar(
                out=ACC[:, :], in0=XA[:, 0:CW],
                scalar1=WB[:, 0:1], scalar2=WB[:, 4:5],
                op0=mult, op1=add)
            nc.vector.scalar_tensor_tensor(
                out=ACC[:, :], in0=XB[:, 0:CW], scalar=WB[:, 2:3],
                in1=ACC[:, :], op0=mult, op1=add)
            nc.vector.scalar_tensor_tensor(
                out=ACC[:, :], in0=XA[:, 1:CW + 1], scalar=WB[:, 1:2],
                in1=ACC[:, :], op0=mult, op1=add)
            nc.vector.scalar_tensor_tensor(
                out=ACC[:, :], in0=XB[:, 1:CW + 1], scalar=WB[:, 3:4],
                in1=ACC[:, :], op0=mult, op1=add).then_inc(vsem, 1)
            # per-partition (per-row) min of logit+b for the guard; runs in
            # the shadow of ACT's sigmoid, off the critical path.
            nc.vector.tensor_reduce(
                out=OUT[0:HO, 0, 0, 1:2], in_=ACC[:, :],
                axis=mybir.AxisListType.X,
                op=mybir.AluOpType.min).then_inc(vsem, 1)

        @block.scalar
        def _(scalar):
            # f = sigmoid(acc) with fused per-partition sum into OUT[:, 0]
            nc.scalar.activation(
                out=F[:, :], in_=ACC[:, :],
                func=mybir.ActivationFunctionType.Sigmoid,
                bias=0.0, scale=1.0,
                accum_out=OUT[0:HO, 0, 0, 0:1],
            )._wait_ge(vsem, 1).then_inc(asem, 1)

        @block.gpsimd
        def _(gp):
            # kv_writeback's Q7 desc-gen code lives in the "attn" ucode
            # library; load it before the prep (off the critical path).
            from concourse import library_config
            nc.gpsimd.load_library(library_config.attn)
            nc.gpsimd.memset(IDX[:, :], 0)
            # Prepared store o[0,p,0,:] = OUT[p,:] (batch=1, d_head=128,
            # ncn=2, ctx_idx=0): descriptors generated here at t~0, fired by
            # trigger_dma below once the results land.
            nc.gpsimd.kv_writeback(
                out_ap=o[:, :, :, :],
                in_ap=OUT[:, :, :, :],
                ctx_idxs_ap=IDX[:, :],
                prepare_only=True,
                sem=ksem,
            ).then_inc(psem, 1)
            gp.wait_ge(psem, 1)    # descriptors committed to the ring
            gp.wait_ge(vsem, 2)    # DVE min landed in OUT[:, 1]
            nc.gpsimd.trigger_dma(count=1)._wait_ge(asem, 1)

    return nc


def _strip_init_barrier(nc):
    """Post-trace edits.

    1. Bass.__init__ emits const-AP memsets plus an all-engine barrier
       before the kernel body. Nothing here reads the const APs and all
       cross-engine ordering is explicit semaphores, so drop the barrier
       (Drain + EventSemaphore per engine).
    2. Hoist the input DMACopy ahead of SP's prelude RegisterMoves
       (zero/bounds-reg init). The DMA references no registers, so the
       moves can run during the transfer instead of serializing ~250ns
       before it on the critical path.
    3. Drop the Block-exit all-engine barrier (the *_end block). SP's
       final ksem>=16 wait already guarantees the output DMA landed and
       every producer finished before the NEFF completes.
    4. Move SP's final ksem wait past its branch, into the end block —
       otherwise the 50ns branch retires after the wait and tail-pads
       the kernel."""
    blocks = nc.m.functions[0].blocks
    bb0 = blocks[0]
    keep, removed = [], []
    for ins in bb0.instructions:
        tn = type(ins).__name__
        if "Drain" in tn or "EventSemaphore" in tn or \
                ins.name.startswith("barrier_"):
            removed.append(ins.name)
            continue
        keep.append(ins)
    assert len(removed) >= 10, removed   # 5 engines x (drain + evsem)

    in_dma = None
    for blk in blocks[1:]:
        for ins in blk.instructions:
            if "DMACopy" in type(ins).__name__:
                src = ins.ins[0]
                if getattr(src, "memref", "") == "pk":
                    in_dma = ins
                    blk.instructions = [
                        i for i in blk.instructions if i.name != ins.name]
                    break
        if in_dma is not None:
            break
    assert in_dma is not None, "input DMACopy not found"
    # index 0 is the pseudo Call; engines only order among their own stream
    bb0.instructions = keep[:1] + [in_dma] + keep[1:]

    end_blk = None
    for blk in blocks:
        if blk.name.endswith("_end"):
            assert all(
                "Drain" in type(i).__name__ or
                "EventSemaphore" in type(i).__name__
                for i in blk.instructions), [
                    type(i).__name__ for i in blk.instructions]
            blk.instructions = []
            end_blk = blk
    assert end_blk is not None, "Block end bb not found"

    for blk in blocks:
        if "_SP_" not in blk.name:
            continue
        insts = blk.instructions
        assert "EventSemaphore" in type(insts[-2]).__name__ and \
            "UnconditionalBranch" in type(insts[-1]).__name__, [
                type(i).__name__ for i in insts]
        w = insts[-2].sync_info.on_wait
        assert len(w) == 1 and w[0].wait_value == 16, w
        final_wait = insts[-2]
        blk.instructions = insts[:-2] + insts[-1:]
        end_blk.instructions = [final_wait]
        break
    else:
        raise AssertionError("SP body block not found")


def _in_maps(x, W, b):
    wb_row = np.concatenate([W.reshape(-1), b.reshape(-1)]).astype(np.float32)
    maps = []
    for c in range(N_CORES):
        bi, s = c // 4, STARTS[c % 4]
        pk = np.empty((HO, PKW), dtype=np.float32)
        pk[:, 0:25] = x[bi, 0, 0:HO, s:s + 25]
        pk[:, 25:50] = x[bi, 0, 1:HI, s:s + 25]
        pk[:, 50:55] = wb_row[None, :]
        maps.append({"pk": pk})
    return maps


def _run_device(x, W, b, trace=False, **kw):
    nc = _build_bass()
    res = run_bass_kernel_spmd(
        nc, _in_maps(x, W, b), core_ids=list(range(N_CORES)), trace=trace, **kw
    )
    return res


def _dup_col_sigmoid_sum(x, W, b, bi):
    """Host f64 sigmoid-sum of the doubly-counted column (STARTS[3])."""
    c = STARTS[3]
    W4 = W.reshape(-1).astype(np.float64)
    col = x[bi, 0, :, c].astype(np.float64)
    col1 = x[bi, 0, :, c + 1].astype(np.float64)
    logit = (W4[0] * col[:-1] + W4[1] * col1[:-1]
             + W4[2] * col[1:] + W4[3] * col1[1:]) + float(b[0])
    return float((1.0 / (1.0 + np.exp(-logit))).sum())


def _combine(results, x, W, b):
    """results: 8 dicts of o [1,128,1,2] -> ([B,1] out, global min logit+b)."""
    out = np.zeros((B, 1), dtype=np.float32)
    gmin_acc = np.inf
    for bi in range(B):
        S = 0.0
        for q in range(4):
            r = results[bi * 4 + q]["o"].reshape(128, 2)
            S += float(r[0:HO, 0].astype(np.float64).sum())
            gmin_acc = min(gmin_acc, float(r[0:HO, 1].min()))
        S -= _dup_col_sigmoid_sum(x, W, b, bi)   # col 71 counted twice
        out[bi, 0] = np.float32(2.0 * S / L)
    # bias is folded on-device, so the reduce already tracked min(logit+b)
    gmin_f = 1.0 / (1.0 + np.exp(-gmin_acc))
    return out, gmin_f


def _fallback(x, W, b):
    # Exact O(L log L) host evaluation of the reference semantics; only
    # reached if some sigmoid output underflows below GUARD_MIN_F.
    out = np.zeros((B, 1), dtype=np.float32)
    W4 = W.reshape(-1).astype(np.float64)
    for bi in range(B):
        img = x[bi, 0].astype(np.float64)
        acc = (W4[0] * img[:-1, :-1] + W4[1] * img[:-1, 1:]
               + W4[2] * img[1:, :-1] + W4[3] * img[1:, 1:]) + float(b[0])
        f = (1.0 / (1.0 + np.exp(-acc))).reshape(-1)
        nf = f / (f + 1e-12)
        order = np.argsort(nf)
        nf_s, f_s = nf[order], f[order]
        suff_f = np.cumsum(f_s[::-1])[::-1]
        thr = GRAPH_T / nf
        idx = np.searchsorted(nf_s, thr, side="left")
        cnt = (len(f) - idx).astype(np.float64)
        aggs = np.where(idx < len(f), suff_f[np.minimum(idx, len(f) - 1)], 0.0)
        self_in = nf * nf >= GRAPH_T
        cnt = cnt - self_in
        aggs = aggs - np.where(self_in, f, 0.0)
        node = f + np.where(cnt > 0, aggs / np.maximum(cnt, 1), 0.0)
        out[bi, 0] = np.float32(node.mean())
    return out


def kernel(x, W, b):
    x = np.ascontiguousarray(np.asarray(x, dtype=np.float32))
    W = np.asarray(W, dtype=np.float32)
    b = np.asarray(b, dtype=np.float32)
    res = _run_device(x, W, b, trace=False)
    out, gmin = _combine(res.results, x, W, b)
    if not (gmin >= GUARD_MIN_F):
        return _fallback(x, W, b)
    return out
